# revision 3
# baseline (speedup 1.0000x reference)
"""Self-contained TRN2 Bass kernel for the RGCN message-passing problem.

Edge-parallel dst-window sharding across 8 cores; all FLOPs on device.
Host prep is pure relayout: edges sorted by dst window and dealt to cores;
per-edge rows packed as [h_src(256) | cw0..2 | dstloc] bf16 (520B/edge) and
streamed densely.

Device pipeline:
- Rows stream in via HWDGE, two quads (8 windows, ~19 chunks) per DMA.
- Per quad (4 windows) DVE builds W[e,(b,d)] = (iota[d]==dstloc[e])*cw[e,b]
  in two broadcast ops.
- Stage 1 per window: Z[f,(b,d)] accumulates over 128-edge chunks in a
  2-bank PSUM tile (h-half per bank); evacuated bf16 to SBUF by the scalar
  engine (7 of 8 windows) or DVE (1 of 8).
- Stage 2 per 16 windows: out.T[o,d] = sum_{b,f} bases[b,f,o] Z[f,(b,d)],
  bases slices stationary (128x128, FWL), moving operand N=512.  ReLU+bias
  on the scalar engine (bias is per-partition in the transposed layout).
"""

import numpy as np
import ml_dtypes

import concourse.bass as bass
import concourse.tile as tile
from concourse import bacc, mybir

F = 256      # in features
O = 256      # out features
NB = 3       # bases
WINDOW = 32  # dst rows per window
QUAD = 4     # windows per W-build batch
DUO = 8      # windows per feature-stream DMA
GRP = 16     # windows per stage-2 group
QCAP = 12    # chunk capacity of a quad
DCAP = 24    # chunk capacity of a duo
GBUFS = 5    # feature-stream (duo) tile buffering depth
RW = 260     # row width (i16): [features x256, cw0, cw1, cw2, dstloc]
WARMUP_MM = 72


def build_program(n_nodes, slot_cws, n_cores=8):
    slot_cws = list(slot_cws)
    nw = len(slot_cws)
    assert nw % GRP == 0
    nchunks = sum(slot_cws)
    dcore = nw * WINDOW

    bf16 = mybir.dt.bfloat16
    f32 = mybir.dt.float32
    i16 = mybir.dt.int16

    # duo (2-quad) partitioning of slots
    duos = []
    qb = 0
    for q0 in range(0, nw, DUO):
        cws = slot_cws[q0:q0 + DUO]
        cwg = sum(cws)
        assert cwg <= DCAP
        duos.append((q0, qb, cws, cwg))
        qb += cwg
    assert qb == nchunks

    nc = bacc.Bacc("TRN2", target_bir_lowering=False, debug=False,
                   num_devices=n_cores)
    hg_d = nc.dram_tensor("hg", [128, nchunks, RW], i16,
                          kind="ExternalInput").ap()
    bases_d = nc.dram_tensor("bases", [NB, F, O], i16,
                             kind="ExternalInput").ap()
    iota_d = nc.dram_tensor("iota", [128, WINDOW], i16,
                            kind="ExternalInput").ap()
    bias_d = nc.dram_tensor("bias", [128, 2], i16, kind="ExternalInput").ap()
    out_d = nc.dram_tensor("out", [2, 128, dcore], i16,
                           kind="ExternalOutput").ap()

    with tile.TileContext(nc) as tc:
        with (
            tc.tile_pool(name="const", bufs=1) as cpool,
            tc.tile_pool(name="feat", bufs=GBUFS) as gpool,
            tc.tile_pool(name="wmat", bufs=4) as wpool,
            tc.tile_pool(name="mask", bufs=4) as mpool,
            tc.tile_pool(name="abt", bufs=2) as apool,
            tc.tile_pool(name="ost", bufs=3) as opool,
            tc.tile_pool(name="ps1", bufs=3, space="PSUM") as ps1,
            tc.tile_pool(name="ps2", bufs=1, space="PSUM") as ps2,
        ):
            # ---- prologue ----
            iota_sb = cpool.tile([128, WINDOW], i16)
            nc.sync.dma_start(iota_sb[:], iota_d[:])

            # HAM warmup on a memset operand (no DMA dependency): keep the
            # PE busy from t~0 so it is at full clock for the real matmuls
            ones_sb = cpool.tile([128, WINDOW], bf16)
            nc.vector.memset(ones_sb[:], 1.0)
            pw = ps1.tile([128, 2, 512], f32, tag="p1", name="pwarm")
            for _ in range(WARMUP_MM):
                nc.tensor.matmul(pw[0:WINDOW, 0, 0:WINDOW],
                                 ones_sb[:], ones_sb[:],
                                 start=True, stop=True)

            bases_i = cpool.tile([128, NB, 2, 2, 128], i16)
            for b in range(NB):
                for h in range(2):
                    for oh in range(2):
                        nc.gpsimd.dma_start(
                            bases_i[:, b, h, oh, :],
                            bases_d[b, h * 128:(h + 1) * 128,
                                    oh * 128:(oh + 1) * 128])
            bias_sb = cpool.tile([128, 2], i16)
            nc.gpsimd.dma_start(bias_sb[:], bias_d[:])

            # ---- main pipeline ----
            abt_of = {}

            def emit_stage2(g, w0=0, w1=GRP, pop=True):
                abt = abt_of[g]
                if pop:
                    del abt_of[g]
                nwin = w1 - w0
                for oh in range(2):
                    p2 = ps2.tile([128, GRP * WINDOW], f32,
                                  tag=f"p2o{oh}", name=f"p2o{oh}")
                    for b in range(NB):
                        for h in range(2):
                            nc.tensor.matmul(
                                p2[0:128, 0:nwin * WINDOW],
                                bases_i[:, b, h, oh, :].bitcast(bf16),
                                abt[:, w0:w1, h,
                                    b * WINDOW:(b + 1) * WINDOW],
                                start=(b == 0 and h == 0),
                                stop=(b == NB - 1 and h == 1))
                    osb = opool.tile([128, GRP * WINDOW], bf16,
                                     tag="osb", name="osb")
                    nc.scalar.activation(
                        osb[0:128, 0:nwin * WINDOW],
                        p2[0:128, 0:nwin * WINDOW],
                        mybir.ActivationFunctionType.Relu,
                        bias=bias_sb[:, oh:oh + 1].bitcast(bf16))
                    nc.gpsimd.dma_start(
                        out_d[oh, :, (g * GRP + w0) * WINDOW:
                              (g * GRP + w1) * WINDOW],
                        osb[0:128, 0:nwin * WINDOW].bitcast(i16))

            for di, (q0, qb, cws, cwg) in enumerate(duos):
                G = gpool.tile([128, DCAP, RW], i16, tag="G", name="G")
                if di == 0:
                    # split the first transfer so quad 0 lands ASAP
                    h0 = sum(cws[0:QUAD])
                    nc.sync.dma_start(G[:, 0:h0, :], hg_d[:, qb:qb + h0, :])
                    nc.sync.dma_start(G[:, h0:cwg, :],
                                      hg_d[:, qb + h0:qb + cwg, :])
                else:
                    nc.sync.dma_start(G[:, 0:cwg, :], hg_d[:, qb:qb + cwg, :])

                c0 = 0
                for half in range(2):
                    hw0 = q0 + half * QUAD
                    hcws = cws[half * QUAD:(half + 1) * QUAD]
                    hcwg = sum(hcws)
                    cq = c0  # chunk offset of this quad within the duo tile

                    # on-chip W build for the quad
                    mask = mpool.tile([128, QCAP, WINDOW], i16, tag="M",
                                      name="M")
                    dst_ap = G[:, cq:cq + hcwg, 259:260].bitcast(bf16) \
                        .broadcast_to([128, hcwg, WINDOW])
                    iota_ap = iota_sb[:].bitcast(bf16).unsqueeze(1) \
                        .broadcast_to([128, hcwg, WINDOW])
                    nc.vector.tensor_tensor(
                        mask[:, 0:hcwg, :].bitcast(bf16), iota_ap, dst_ap,
                        mybir.AluOpType.is_equal)
                    W = wpool.tile([128, QCAP, NB, WINDOW], i16, tag="W",
                                   name="W")
                    cw_ap = G[:, cq:cq + hcwg, 256:256 + NB].bitcast(bf16) \
                        .unsqueeze(3).broadcast_to([128, hcwg, NB, WINDOW])
                    mask_ap = mask[:, 0:hcwg, :].bitcast(bf16).unsqueeze(2) \
                        .broadcast_to([128, hcwg, NB, WINDOW])
                    nc.vector.tensor_tensor(
                        W[:, 0:hcwg, :, :].bitcast(bf16), mask_ap, cw_ap,
                        mybir.AluOpType.mult)

                    # stage 1 over the quad's 4 windows
                    for j, cw in enumerate(hcws):
                        i = hw0 + j       # global window slot
                        pos = i % GRP
                        # 2-bank psum tile per window: h-half h in bank h
                        p1 = ps1.tile([128, 2, 512], f32, tag="p1",
                                      name="p1")
                        for c in range(c0, c0 + cw):
                            for h in range(2):
                                nc.tensor.matmul(
                                    p1[:, h, 0:NB * WINDOW],
                                    G[:, c, h * 128:(h + 1) * 128]
                                    .bitcast(bf16),
                                    W[:, c - cq, :, :].bitcast(bf16),
                                    start=(c == c0), stop=(c == c0 + cw - 1))
                        c0 += cw

                        # evacuate Z (both h-halves, one op); DVE takes
                        # 1 in 8, and every other one in the final group
                        if pos == 0:
                            abt_of[i // GRP] = apool.tile(
                                [128, GRP, 2, NB * WINDOW], bf16,
                                tag="abt", name="abt")
                        abt_all = abt_of[i // GRP]
                        if (i % 8 == 3 or
                                (i >= nw - GRP and i % 2 == 1)):
                            nc.vector.tensor_copy(
                                abt_all[:, pos, :, :],
                                p1[:, :, 0:NB * WINDOW])
                        else:
                            nc.scalar.activation(
                                abt_all[:, pos, :, :],
                                p1[:, :, 0:NB * WINDOW],
                                mybir.ActivationFunctionType.Copy)

                        # stage 2 for group g-1, deferred 3 windows so the
                        # new group's evacuations queue ahead of the ReLU
                        if pos == 2 and i >= GRP:
                            emit_stage2(i // GRP - 1)
                        # first half of the final group as soon as its
                        # windows are evacuated, to shorten the tail
                        if i == nw - GRP + 10:
                            emit_stage2(nw // GRP - 1, 0, 8, pop=False)

            emit_stage2(nw // GRP - 1, 8, GRP)

    nc.compile()
    return nc


def host_prep(h_bf, src, dst, rel, comp, n_nodes, n_cores):
    """Sort/deal/pad edges; pre-gather features into packed dense rows."""
    dcore = n_nodes // n_cores
    nw = dcore // WINDOW
    ngw = n_cores * nw
    w_edge = comp[rel].astype(ml_dtypes.bfloat16)        # [E, NB]
    gw = (dst // WINDOW).astype(np.int64)
    order = np.argsort(gw, kind="stable")
    counts = np.bincount(gw, minlength=ngw)
    starts = np.concatenate([[0], np.cumsum(counts)])

    # deal windows to cores by descending count; slot capacity = group max
    ranked = np.argsort(-counts, kind="stable")
    # lightest rank-groups first: faster pipeline ramp, heavier windows run
    # at full lookahead
    ranked = ranked.reshape(-1, n_cores)[::-1].reshape(-1)
    slot_cws = [max(1, -(-int(counts[ranked[n_cores * i]]) // 128))
                for i in range(nw)]
    nchunks = sum(slot_cws)
    epad = nchunks * 128

    gidx = np.zeros((n_cores, epad), np.int64)
    valid = np.zeros((n_cores, epad), bool)
    meta = np.zeros((n_cores, epad, RW - F), np.int16)
    win_of_slot = np.zeros((n_cores, nw), np.int64)
    dstloc_bf = (dst % WINDOW).astype(ml_dtypes.bfloat16).view(np.int16)

    slot_base = np.zeros(nw, np.int64)
    acc = 0
    for i, cwv in enumerate(slot_cws):
        slot_base[i] = acc
        acc += cwv
    for k in range(n_cores):
        for i in range(nw):
            wid = int(ranked[n_cores * i + k])
            win_of_slot[k, i] = wid
            es = order[starts[wid]:starts[wid + 1]]
            base = int(slot_base[i]) * 128
            n = len(es)
            gidx[k, base:base + n] = src[es]
            valid[k, base:base + n] = True
            meta[k, base:base + n, 0:NB] = w_edge[es].view(np.int16)
            meta[k, base:base + n, NB] = dstloc_bf[es]

    # dense packed rows [features | cw | dstloc], edge e -> [e%128, e//128]
    hg = np.zeros((n_cores, epad, RW), np.int16)
    hg[:, :, 0:F] = h_bf[gidx.reshape(-1)].reshape(n_cores, epad, F)
    hg[~valid, 0:F] = 0
    hg[:, :, F:RW] = meta
    hg = np.ascontiguousarray(
        hg.reshape(n_cores, nchunks, 128, RW).transpose(0, 2, 1, 3))
    return hg, tuple(slot_cws), win_of_slot


def rgcn_kernel(text, src, dst, rel, bases, comp, bias, n_cores=8,
                run_fn=None, nc_cache={}):
    """Full-input kernel: shard, run on 8 cores, reassemble output."""
    Bt, St, INF = text.shape
    n_nodes = Bt * St
    h = text.reshape(n_nodes, INF)

    src = np.asarray(src).astype(np.int64)
    dst = np.asarray(dst).astype(np.int64)
    rel = np.asarray(rel).astype(np.int64)
    bases_np = np.asarray(bases, np.float32)
    comp_np = np.asarray(comp, np.float32)
    bias_np = np.asarray(bias, np.float32)

    h_bf = np.asarray(h, np.float32).astype(ml_dtypes.bfloat16).view(np.int16)
    hg, slot_cws, win_of_slot = host_prep(
        h_bf, src, dst, rel, comp_np, n_nodes, n_cores)
    key = (n_nodes, slot_cws, n_cores)
    if key not in nc_cache:
        nc_cache[key] = build_program(n_nodes, slot_cws, n_cores)
    nc = nc_cache[key]

    bases_bf = bases_np.astype(ml_dtypes.bfloat16).view(np.int16)
    # bias2[p, oh] = bias[oh*128 + p]
    bias_bf = bias_np.astype(ml_dtypes.bfloat16).view(np.int16) \
        .reshape(2, 128).T.copy()
    iota_bc = np.broadcast_to(
        np.arange(WINDOW, dtype=np.float32).astype(ml_dtypes.bfloat16)
        .view(np.int16)[None, :], (128, WINDOW)).copy()

    in_maps = [
        dict(hg=hg[k], bases=bases_bf, iota=iota_bc, bias=bias_bf)
        for k in range(n_cores)
    ]
    from concourse.bass_utils import run_bass_kernel_spmd
    if run_fn is None:
        res = run_bass_kernel_spmd(nc, in_maps, list(range(n_cores)))
        outs = [res.results[k]["out"] for k in range(n_cores)]
    else:
        outs = run_fn(nc, in_maps)

    out = np.zeros((n_nodes, O), np.float32)
    nw = len(slot_cws)
    W = WINDOW
    for k in range(n_cores):
        # outs[k]: [2, 128, dcore] (o_half, o_low, d) -> [dcore, 256]
        ok = outs[k].view(ml_dtypes.bfloat16).astype(np.float32)
        ok = ok.transpose(2, 0, 1).reshape(-1, O)
        for i in range(nw):
            wid = win_of_slot[k][i]
            out[wid * W:(wid + 1) * W] = ok[i * W:(i + 1) * W]
    return out.reshape(Bt, St, O)


_NC_CACHE = {}


def kernel(text, src, dst, rel, bases, comp, bias):
    out = rgcn_kernel(
        np.asarray(text, np.float32),
        np.asarray(src), np.asarray(dst), np.asarray(rel),
        np.asarray(bases, np.float32), np.asarray(comp, np.float32),
        np.asarray(bias, np.float32),
        n_cores=8, nc_cache=_NC_CACHE)
    return np.ascontiguousarray(out, np.float32)


# revision 5
# speedup vs baseline: 1.2056x; 1.2056x over previous
"""Self-contained TRN2 Bass kernel for the RGCN message-passing problem.

Edge-parallel dst-window sharding across 8 cores; all FLOPs on device.
Host prep is pure relayout: edges sorted by dst window and dealt to cores;
per-edge rows packed as [h_src(256) | cw0..2 | dstloc] bf16 (520B/edge) and
streamed densely.

Device pipeline:
- Rows stream in via HWDGE, two quads (8 windows, ~19 chunks) per DMA.
- Per quad (4 windows) DVE builds W[e,(b,d)] = (iota[d]==dstloc[e])*cw[e,b]
  in two broadcast ops.
- Stage 1 per window: Z[f,(b,d)] accumulates over 128-edge chunks in a
  2-bank PSUM tile (h-half per bank); evacuated bf16 to SBUF by the scalar
  engine (7 of 8 windows) or DVE (1 of 8).
- Stage 2 per 16 windows: out.T[o,d] = sum_{b,f} bases[b,f,o] Z[f,(b,d)],
  bases slices stationary (128x128, FWL), moving operand N=512.  ReLU+bias
  on the scalar engine (bias is per-partition in the transposed layout).
"""

import numpy as np
import ml_dtypes

import concourse.bass as bass
import concourse.tile as tile
from concourse import bacc, mybir

F = 256      # in features
O = 256      # out features
NB = 3       # bases
WINDOW = 32  # dst rows per window
QUAD = 4     # windows per W-build batch
DUO = 8      # windows per feature-stream DMA
GRP = 16     # windows per stage-2 group
QCAP = 12    # chunk capacity of a quad
DCAP = 24    # chunk capacity of a duo
GBUFS = 6    # feature-stream (duo) tile buffering depth
RW = 256     # feature row width (i16); meta is a separate tensor
WARMUP_MM = 72


def build_program(n_nodes, slot_cws, n_cores=8):
    slot_cws = list(slot_cws)
    nw = len(slot_cws)
    assert nw % GRP == 0
    nchunks = sum(slot_cws)
    dcore = nw * WINDOW

    bf16 = mybir.dt.bfloat16
    f32 = mybir.dt.float32
    i16 = mybir.dt.int16

    # duo (2-quad) partitioning of slots
    duos = []
    qb = 0
    for q0 in range(0, nw, DUO):
        cws = slot_cws[q0:q0 + DUO]
        cwg = sum(cws)
        assert cwg <= DCAP
        duos.append((q0, qb, cws, cwg))
        qb += cwg
    assert qb == nchunks

    nc = bacc.Bacc("TRN2", target_bir_lowering=False, debug=False,
                   num_devices=n_cores)
    hg_d = nc.dram_tensor("hg", [128, nchunks, RW], i16,
                          kind="ExternalInput").ap()
    meta_d = nc.dram_tensor("meta", [128, nchunks, 4], i16,
                            kind="ExternalInput").ap()
    bases_d = nc.dram_tensor("bases", [NB, F, O], i16,
                             kind="ExternalInput").ap()
    iota_d = nc.dram_tensor("iota", [128, WINDOW], i16,
                            kind="ExternalInput").ap()
    bias_d = nc.dram_tensor("bias", [128, 2], i16, kind="ExternalInput").ap()
    out_d = nc.dram_tensor("out", [2, 128, dcore], i16,
                           kind="ExternalOutput").ap()

    with tile.TileContext(nc) as tc:
        with (
            tc.tile_pool(name="const", bufs=1) as cpool,
            tc.tile_pool(name="feat", bufs=GBUFS) as gpool,
            tc.tile_pool(name="wmat", bufs=4) as wpool,
            tc.tile_pool(name="mask", bufs=4) as mpool,
            tc.tile_pool(name="abt", bufs=2) as apool,
            tc.tile_pool(name="ost", bufs=3) as opool,
            tc.tile_pool(name="ps1", bufs=3, space="PSUM") as ps1,
            tc.tile_pool(name="ps2", bufs=1, space="PSUM") as ps2,
        ):
            # ---- prologue ----
            iota_sb = cpool.tile([128, WINDOW], i16)
            nc.sync.dma_start(iota_sb[:], iota_d[:])
            meta_sb = cpool.tile([128, nchunks, 4], i16)
            nc.sync.dma_start(meta_sb[:], meta_d[:])

            # HAM warmup on a memset operand (no DMA dependency): keep the
            # PE busy from t~0 so it is at full clock for the real matmuls
            ones_sb = cpool.tile([128, WINDOW], bf16)
            nc.vector.memset(ones_sb[:], 1.0)
            pw = ps1.tile([128, 2, 512], f32, tag="p1", name="pwarm")
            for _ in range(WARMUP_MM):
                nc.tensor.matmul(pw[0:WINDOW, 0, 0:WINDOW],
                                 ones_sb[:], ones_sb[:],
                                 start=True, stop=True)

            bases_i = cpool.tile([128, NB, 2, 2, 128], i16)
            for b in range(NB):
                for h in range(2):
                    for oh in range(2):
                        nc.gpsimd.dma_start(
                            bases_i[:, b, h, oh, :],
                            bases_d[b, h * 128:(h + 1) * 128,
                                    oh * 128:(oh + 1) * 128])
            bias_sb = cpool.tile([128, 2], i16)
            nc.gpsimd.dma_start(bias_sb[:], bias_d[:])

            # ---- main pipeline ----
            abt_of = {}

            def emit_stage2(g, w0=0, w1=GRP, pop=True, store_eng=None):
                abt = abt_of[g]
                if pop:
                    del abt_of[g]
                nwin = w1 - w0
                for oh in range(2):
                    p2 = ps2.tile([128, GRP * WINDOW], f32,
                                  tag=f"p2o{oh}", name=f"p2o{oh}")
                    for b in range(NB):
                        for h in range(2):
                            nc.tensor.matmul(
                                p2[0:128, 0:nwin * WINDOW],
                                bases_i[:, b, h, oh, :].bitcast(bf16),
                                abt[:, w0:w1, h,
                                    b * WINDOW:(b + 1) * WINDOW],
                                start=(b == 0 and h == 0),
                                stop=(b == NB - 1 and h == 1))
                    osb = opool.tile([128, GRP * WINDOW], bf16,
                                     tag="osb", name="osb")
                    nc.scalar.activation(
                        osb[0:128, 0:nwin * WINDOW],
                        p2[0:128, 0:nwin * WINDOW],
                        mybir.ActivationFunctionType.Relu,
                        bias=bias_sb[:, oh:oh + 1].bitcast(bf16))
                    (store_eng or nc.gpsimd).dma_start(
                        out_d[oh, :, (g * GRP + w0) * WINDOW:
                              (g * GRP + w1) * WINDOW],
                        osb[0:128, 0:nwin * WINDOW].bitcast(i16))

            for di, (q0, qb, cws, cwg) in enumerate(duos):
                G = gpool.tile([128, DCAP, RW], i16, tag="G", name="G")
                # one transfer per quad: the first quad's W-build starts
                # without waiting for the whole duo
                h0 = sum(cws[0:QUAD])
                nc.sync.dma_start(G[:, 0:h0, :], hg_d[:, qb:qb + h0, :])
                nc.sync.dma_start(G[:, h0:cwg, :],
                                  hg_d[:, qb + h0:qb + cwg, :])

                c0 = 0
                for half in range(2):
                    hw0 = q0 + half * QUAD
                    hcws = cws[half * QUAD:(half + 1) * QUAD]
                    hcwg = sum(hcws)
                    cq = c0  # chunk offset of this quad within the duo tile

                    # on-chip W build for the quad
                    mask = mpool.tile([128, QCAP, WINDOW], i16, tag="M",
                                      name="M")
                    dst_ap = meta_sb[:, qb + cq:qb + cq + hcwg, 3:4] \
                        .bitcast(bf16).broadcast_to([128, hcwg, WINDOW])
                    iota_ap = iota_sb[:].bitcast(bf16).unsqueeze(1) \
                        .broadcast_to([128, hcwg, WINDOW])
                    nc.vector.tensor_tensor(
                        mask[:, 0:hcwg, :].bitcast(bf16), iota_ap, dst_ap,
                        mybir.AluOpType.is_equal)
                    W = wpool.tile([128, QCAP, NB, WINDOW], i16, tag="W",
                                   name="W")
                    cw_ap = meta_sb[:, qb + cq:qb + cq + hcwg, 0:NB] \
                        .bitcast(bf16) \
                        .unsqueeze(3).broadcast_to([128, hcwg, NB, WINDOW])
                    mask_ap = mask[:, 0:hcwg, :].bitcast(bf16).unsqueeze(2) \
                        .broadcast_to([128, hcwg, NB, WINDOW])
                    nc.vector.tensor_tensor(
                        W[:, 0:hcwg, :, :].bitcast(bf16), mask_ap, cw_ap,
                        mybir.AluOpType.mult)

                    # stage 1 over the quad's 4 windows
                    for j, cw in enumerate(hcws):
                        i = hw0 + j       # global window slot
                        pos = i % GRP
                        # 2-bank psum tile per window: h-half h in bank h
                        p1 = ps1.tile([128, 2, 512], f32, tag="p1",
                                      name="p1")
                        for c in range(c0, c0 + cw):
                            for h in range(2):
                                nc.tensor.matmul(
                                    p1[:, h, 0:NB * WINDOW],
                                    G[:, c, h * 128:(h + 1) * 128]
                                    .bitcast(bf16),
                                    W[:, c - cq, :, :].bitcast(bf16),
                                    start=(c == c0), stop=(c == c0 + cw - 1))
                        c0 += cw

                        # evacuate Z (both h-halves, one op); DVE takes
                        # 1 in 8, and every other one in the final group
                        if pos == 0:
                            abt_of[i // GRP] = apool.tile(
                                [128, GRP, 2, NB * WINDOW], bf16,
                                tag="abt", name="abt")
                        abt_all = abt_of[i // GRP]
                        if (i % 8 == 7 or
                                (i >= nw - GRP and i % 2 == 1)):
                            nc.vector.tensor_copy(
                                abt_all[:, pos, :, :],
                                p1[:, :, 0:NB * WINDOW])
                        else:
                            nc.scalar.activation(
                                abt_all[:, pos, :, :],
                                p1[:, :, 0:NB * WINDOW],
                                mybir.ActivationFunctionType.Copy)

                        # stage 2 for group g-1, deferred 3 windows so the
                        # new group's evacuations queue ahead of the ReLU
                        if pos == 2 and i >= GRP:
                            emit_stage2(i // GRP - 1)
                        # first half of the final group as soon as its
                        # windows are evacuated, to shorten the tail
                        if i == nw - GRP + 10:
                            emit_stage2(nw // GRP - 1, 0, 8, pop=False)

            emit_stage2(nw // GRP - 1, 8, GRP)

    nc.compile()
    return nc


def host_prep(h_bf, src, dst, rel, comp, n_nodes, n_cores):
    """Sort/deal/pad edges; pre-gather features into packed dense rows."""
    dcore = n_nodes // n_cores
    nw = dcore // WINDOW
    ngw = n_cores * nw
    w_edge = comp[rel].astype(ml_dtypes.bfloat16)        # [E, NB]
    gw = (dst // WINDOW).astype(np.int64)
    order = np.argsort(gw, kind="stable")
    counts = np.bincount(gw, minlength=ngw)
    starts = np.concatenate([[0], np.cumsum(counts)])

    # deal windows to cores by descending count; slot capacity = group max
    ranked = np.argsort(-counts, kind="stable")
    # lightest rank-groups first: faster pipeline ramp, heavier windows run
    # at full lookahead
    ranked = ranked.reshape(-1, n_cores)[::-1].reshape(-1)
    slot_cws = [max(1, -(-int(counts[ranked[n_cores * i]]) // 128))
                for i in range(nw)]
    nchunks = sum(slot_cws)
    epad = nchunks * 128

    gidx = np.zeros((n_cores, epad), np.int64)
    valid = np.zeros((n_cores, epad), bool)
    meta = np.zeros((n_cores, epad, 4), np.int16)
    win_of_slot = np.zeros((n_cores, nw), np.int64)
    dstloc_bf = (dst % WINDOW).astype(ml_dtypes.bfloat16).view(np.int16)

    slot_base = np.zeros(nw, np.int64)
    acc = 0
    for i, cwv in enumerate(slot_cws):
        slot_base[i] = acc
        acc += cwv
    for k in range(n_cores):
        for i in range(nw):
            wid = int(ranked[n_cores * i + k])
            win_of_slot[k, i] = wid
            es = order[starts[wid]:starts[wid + 1]]
            base = int(slot_base[i]) * 128
            n = len(es)
            gidx[k, base:base + n] = src[es]
            valid[k, base:base + n] = True
            meta[k, base:base + n, 0:NB] = w_edge[es].view(np.int16)
            meta[k, base:base + n, NB] = dstloc_bf[es]

    # dense feature rows, edge e -> [e%128, e//128]; meta separate
    hg = h_bf[gidx.reshape(-1)].reshape(n_cores, epad, F).copy()
    hg[~valid] = 0
    hg = np.ascontiguousarray(
        hg.reshape(n_cores, nchunks, 128, F).transpose(0, 2, 1, 3))
    meta_t = np.ascontiguousarray(
        meta.reshape(n_cores, nchunks, 128, 4).transpose(0, 2, 1, 3))
    return hg, meta_t, tuple(slot_cws), win_of_slot


def rgcn_kernel(text, src, dst, rel, bases, comp, bias, n_cores=8,
                run_fn=None, nc_cache={}):
    """Full-input kernel: shard, run on 8 cores, reassemble output."""
    Bt, St, INF = text.shape
    n_nodes = Bt * St
    h = text.reshape(n_nodes, INF)

    src = np.asarray(src).astype(np.int64)
    dst = np.asarray(dst).astype(np.int64)
    rel = np.asarray(rel).astype(np.int64)
    bases_np = np.asarray(bases, np.float32)
    comp_np = np.asarray(comp, np.float32)
    bias_np = np.asarray(bias, np.float32)

    h_bf = np.asarray(h, np.float32).astype(ml_dtypes.bfloat16).view(np.int16)
    hg, meta_t, slot_cws, win_of_slot = host_prep(
        h_bf, src, dst, rel, comp_np, n_nodes, n_cores)
    key = (n_nodes, slot_cws, n_cores)
    if key not in nc_cache:
        nc_cache[key] = build_program(n_nodes, slot_cws, n_cores)
    nc = nc_cache[key]

    bases_bf = bases_np.astype(ml_dtypes.bfloat16).view(np.int16)
    # bias2[p, oh] = bias[oh*128 + p]
    bias_bf = bias_np.astype(ml_dtypes.bfloat16).view(np.int16) \
        .reshape(2, 128).T.copy()
    iota_bc = np.broadcast_to(
        np.arange(WINDOW, dtype=np.float32).astype(ml_dtypes.bfloat16)
        .view(np.int16)[None, :], (128, WINDOW)).copy()

    in_maps = [
        dict(hg=hg[k], meta=meta_t[k],
             bases=bases_bf, iota=iota_bc, bias=bias_bf)
        for k in range(n_cores)
    ]
    from concourse.bass_utils import run_bass_kernel_spmd
    if run_fn is None:
        res = run_bass_kernel_spmd(nc, in_maps, list(range(n_cores)))
        outs = [res.results[k]["out"] for k in range(n_cores)]
    else:
        outs = run_fn(nc, in_maps)

    out = np.zeros((n_nodes, O), np.float32)
    nw = len(slot_cws)
    W = WINDOW
    for k in range(n_cores):
        # outs[k]: [2, 128, dcore] (o_half, o_low, d) -> [dcore, 256]
        ok = outs[k].view(ml_dtypes.bfloat16).astype(np.float32)
        ok = ok.transpose(2, 0, 1).reshape(-1, O)
        for i in range(nw):
            wid = win_of_slot[k][i]
            out[wid * W:(wid + 1) * W] = ok[i * W:(i + 1) * W]
    return out.reshape(Bt, St, O)


_NC_CACHE = {}


def kernel(text, src, dst, rel, bases, comp, bias):
    out = rgcn_kernel(
        np.asarray(text, np.float32),
        np.asarray(src), np.asarray(dst), np.asarray(rel),
        np.asarray(bases, np.float32), np.asarray(comp, np.float32),
        np.asarray(bias, np.float32),
        n_cores=8, nc_cache=_NC_CACHE)
    return np.ascontiguousarray(out, np.float32)
